# revision 1
# baseline (speedup 1.0000x reference)
"""Trainium2 Bass kernel for nn_Attention_11106785428044.

Math (reference, per head h and batch b):
  f_x = LN(x) @ w_in (x in {q,k,v}), heads of size DH=64
  dots = (1-w)*cos_sim(f_q,f_k) + w*cov(f_q,f_k);  out = dots @ f_v; proj w_out
The attention matrix is used LINEARLY (no softmax), so with
  qhat = f_q/|f_q|, qc = f_q - mean_d(f_q), khat = f_k/|f_k|, kc = f_k - mean_d(f_k)
  dots @ f_v = [qhat, qc] @ G,  G = [(1-w)*khat, (w/DH)*kc]^T @ f_v   (G is 128x64)
which removes both NxN matrices. The gate w (a scalar per (h,b)) only needs
column-means of LN(q),LN(k) projected through w_in — computed on host.

Sharding: 8 cores = 4 batches x 2 head-groups (8 heads / 512 inner dims each).
Each core computes a [1024,1024] partial of out = Y @ w_out; host sums the two
head-group partials per batch and adds b_out.
"""

import numpy as np
import ml_dtypes

import concourse.bass as bass
import concourse.mybir as mybir
import concourse.tile as tile
from concourse import bacc
from concourse.bass_utils import run_bass_kernel_spmd
from concourse.masks import make_identity

F32 = mybir.dt.float32
BF16 = mybir.dt.bfloat16
AF = mybir.ActivationFunctionType
ALU = mybir.AluOpType

EPS = 1e-5
B, N, DIM = 4, 1024, 1024
H, DH = 16, 64
NT = 8          # token tiles of 128
HG = 8          # heads per core
IG = HG * DH    # inner dims per core = 512
NCORES = 8


def _emit(tc: tile.TileContext, dram):
    nc = tc.nc
    import contextlib
    ctx = contextlib.ExitStack()
    with ctx:
        consts = ctx.enter_context(tc.tile_pool(name="consts", bufs=1))
        xpool = ctx.enter_context(tc.tile_pool(name="x", bufs=3))
        xlnpool = ctx.enter_context(tc.tile_pool(name="xln", bufs=3))
        xtpool = ctx.enter_context(tc.tile_pool(name="xt", bufs=2))
        fpool = ctx.enter_context(tc.tile_pool(name="f", bufs=1))
        ftmp = ctx.enter_context(tc.tile_pool(name="ftmp", bufs=2))
        kqpool = ctx.enter_context(tc.tile_pool(name="kq", bufs=1))
        q2pool = ctx.enter_context(tc.tile_pool(name="q2", bufs=2))
        stat = ctx.enter_context(tc.tile_pool(name="stat", bufs=6))
        gpool = ctx.enter_context(tc.tile_pool(name="g", bufs=1))
        opool = ctx.enter_context(tc.tile_pool(name="o", bufs=2))
        psT = ctx.enter_context(tc.tile_pool(name="psT", bufs=3, space="PSUM"))
        psF = ctx.enter_context(tc.tile_pool(name="psF", bufs=2, space="PSUM"))
        psG = ctx.enter_context(tc.tile_pool(name="psG", bufs=1, space="PSUM"))
        psY = ctx.enter_context(tc.tile_pool(name="psY", bufs=1, space="PSUM"))

        ident = consts.tile([128, 128], BF16)
        make_identity(nc, ident)
        w1_sb = consts.tile([128, 8, IG], BF16)
        nc.sync.dma_start(out=w1_sb, in_=dram["w1"])
        wout_sb = consts.tile([128, 4, DIM], BF16)
        nc.sync.dma_start(out=wout_sb, in_=dram["wout"])
        scal_sb = consts.tile([128, HG], F32)
        nc.sync.dma_start(out=scal_sb, in_=dram["scal"])
        eps_sb = consts.tile([128, 1], F32)
        nc.vector.memset(eps_sb, EPS)

        # upfront cast-loads (SWDGE): each DMA then carries <=1 sync wait
        loaded = {}
        for tag, dname in (("v", "v"), ("k", "k"), ("q", "q")):
            xb = xpool.tile([128, NT, DIM], BF16, tag="xb")
            nc.gpsimd.dma_start(
                out=xb, in_=dram[dname].rearrange("(t p) d -> p t d", p=128)
            )
            loaded[tag] = xb

        def load_ln_transpose(x_dram, tag):
            """cast bf16, LayerNorm rows, PE-transpose to feature-major
            X^T [128 D x (8 chunks) , 1024 tokens]."""
            xb = loaded[tag]
            xt = xtpool.tile([128, 8, N], BF16, tag="xt")
            for t in range(NT):
                st = stat.tile([128, 2, 6], F32, tag="st")
                nc.vector.bn_stats(st[:, 0, :], xb[:, t, 0:512])
                nc.vector.bn_stats(st[:, 1, :], xb[:, t, 512:1024])
                mv = stat.tile([128, 2], F32, tag="mv")
                nc.vector.bn_aggr(mv, st)
                rstd = stat.tile([128, 1], F32, tag="rstd")
                nc.scalar.activation(rstd, mv[:, 1:2], AF.Sqrt, bias=eps_sb)
                nc.vector.reciprocal(rstd, rstd)
                xln = xlnpool.tile([128, DIM], BF16, tag="xlnb")
                nc.vector.tensor_scalar(
                    xln, xb[:, t, :],
                    scalar1=mv[:, 0:1], scalar2=rstd,
                    op0=ALU.subtract, op1=ALU.mult,
                )
                for c in range(8):
                    pt = psT.tile([128, 128], BF16, tag="pt")
                    nc.tensor.transpose(pt, xln[:, c * 128:(c + 1) * 128], ident)
                    nc.scalar.copy(xt[:, c, t * 128:(t + 1) * 128], pt)
            return xt

        def project(xt, t):
            pf = psF.tile([128, IG], F32, tag="pf")
            for c in range(8):
                nc.tensor.matmul(
                    pf, lhsT=xt[:, c, t * 128:(t + 1) * 128], rhs=w1_sb[:, c, :],
                    start=(c == 0), stop=(c == 7),
                )
            return pf

        def head_stats(f_sb, t):
            """per-token, per-head  1/||f|| and mean_d f  from bf16 f."""
            fv3 = f_sb[:, t, :].rearrange("p (h d) -> p h d", h=HG)
            sq = stat.tile([128, HG, DH], F32, tag="sq")
            nc.vector.tensor_mul(sq, fv3, fv3)
            ssq = stat.tile([128, HG], F32, tag="ssq")
            nc.vector.reduce_sum(ssq, sq, axis=mybir.AxisListType.X)
            s = stat.tile([128, HG], F32, tag="s")
            nc.vector.reduce_sum(s, fv3, axis=mybir.AxisListType.X)
            rn = stat.tile([128, HG], F32, tag="rn")
            nc.scalar.activation(rn, ssq, AF.Sqrt)
            nc.vector.reciprocal(rn, rn)
            m = stat.tile([128, HG], F32, tag="m")
            nc.scalar.mul(m, s, 1.0 / DH)
            return rn, m

        def form_pair(dst, f_sb, t, h, rn, m):
            """dst[:, 0:64] = f*rn ; dst[:, 64:128] = f - m   (one head)."""
            fsl = f_sb[:, t, h * DH:(h + 1) * DH]
            nc.gpsimd.tensor_scalar_mul(dst[:, 0:DH], fsl, scalar1=rn[:, h:h + 1])
            nc.gpsimd.tensor_scalar(
                dst[:, DH:2 * DH], fsl, scalar1=m[:, h:h + 1], scalar2=None,
                op0=ALU.subtract,
            )

        # ---- V phase ----
        xt_v = load_ln_transpose(dram["v"], "v")
        f_v = fpool.tile([128, NT, IG], BF16, tag="fv")
        for t in range(NT):
            pf = project(xt_v, t)
            nc.scalar.copy(f_v[:, t, :], pf)

        # ---- K phase ----
        xt_k = load_ln_transpose(dram["k"], "k")
        K2 = kqpool.tile([128, NT, HG, 2 * DH], BF16, tag="K2")
        for t in range(NT):
            pf = project(xt_k, t)
            f_k = ftmp.tile([128, 1, IG], BF16, tag="fk")
            nc.scalar.copy(f_k[:, 0, :], pf)
            rn, m = head_stats(f_k, 0)
            for h in range(HG):
                form_pair(K2[:, t, h, :], f_k, 0, h, rn, m)

        # ---- stage B: G_h = K2_h^T @ f_v_h  (contract tokens), scale rows ----
        G_sb = gpool.tile([128, HG * DH], BF16, tag="G")
        for h in range(HG):
            pg = psG.tile([128, DH], F32, tag="pg")
            for t in range(NT):
                nc.tensor.matmul(
                    pg, lhsT=K2[:, t, h, :], rhs=f_v[:, t, h * DH:(h + 1) * DH],
                    start=(t == 0), stop=(t == NT - 1),
                )
            nc.vector.tensor_scalar_mul(
                G_sb[:, h * DH:(h + 1) * DH], pg, scalar1=scal_sb[:, h:h + 1]
            )

        # ---- Q phase ----
        xt_q = load_ln_transpose(dram["q"], "q")
        QT = kqpool.tile([128, HG, N], BF16, tag="QT")
        for t in range(NT):
            pf = project(xt_q, t)
            f_q = ftmp.tile([128, 1, IG], BF16, tag="fq")
            nc.scalar.copy(f_q[:, 0, :], pf)
            rn, m = head_stats(f_q, 0)
            q2 = q2pool.tile([128, HG, 2 * DH], BF16, tag="q2t")
            for h in range(HG):
                form_pair(q2[:, h, :], f_q, 0, h, rn, m)
            for h in range(HG):
                pt = psT.tile([128, 128], BF16, tag="pt")
                nc.tensor.transpose(pt, q2[:, h, :], ident)
                nc.scalar.copy(QT[:, h, t * 128:(t + 1) * 128], pt)

        # ---- stage C: Y^T_h = G_h^T @ Q''^T_h  -> feature-major Y^T ----
        YT = gpool.tile([128, 4, N], BF16, tag="YT")
        for h in range(HG):
            for blk in range(2):
                py = psY.tile([DH, 512], F32, tag="py")
                nc.tensor.matmul(
                    py, lhsT=G_sb[:, h * DH:(h + 1) * DH],
                    rhs=QT[:, h, blk * 512:(blk + 1) * 512],
                    start=True, stop=True,
                )
                po = (h % 2) * DH
                nc.scalar.copy(YT[po:po + DH, h // 2, blk * 512:(blk + 1) * 512], py)

        # ---- stage D: out = Y @ w_out  (contract inner), token-major out ----
        for t in range(NT):
            out_sb = opool.tile([128, DIM], F32, tag="osb")
            for cb in range(2):
                po = psF.tile([128, 512], F32, tag="pf")
                for j in range(4):
                    nc.tensor.matmul(
                        po, lhsT=YT[:, j, t * 128:(t + 1) * 128],
                        rhs=wout_sb[:, j, cb * 512:(cb + 1) * 512],
                        start=(j == 0), stop=(j == 3),
                    )
                nc.scalar.copy(out_sb[:, cb * 512:(cb + 1) * 512], po)
            nc.scalar.dma_start(
                out=dram["out"][t * 128:(t + 1) * 128, :], in_=out_sb
            )


_CACHE = {}


def _build():
    if "nc" in _CACHE:
        return _CACHE["nc"], _CACHE["names"]
    nc = bacc.Bacc("TRN2", target_bir_lowering=False, debug=False)
    dram = {
        "q": nc.dram_tensor("q", [N, DIM], F32, kind="ExternalInput"),
        "k": nc.dram_tensor("k", [N, DIM], F32, kind="ExternalInput"),
        "v": nc.dram_tensor("v", [N, DIM], F32, kind="ExternalInput"),
        "w1": nc.dram_tensor("w1", [128, 8, IG], BF16, kind="ExternalInput"),
        "wout": nc.dram_tensor("wout", [128, 4, DIM], BF16, kind="ExternalInput"),
        "scal": nc.dram_tensor("scal", [128, HG], F32, kind="ExternalInput"),
        "out": nc.dram_tensor("out", [N, DIM], F32, kind="ExternalOutput"),
    }
    with tile.TileContext(nc) as tc:
        _emit(tc, {k: v[:] for k, v in dram.items()})
    nc.compile()
    _CACHE["nc"] = nc
    _CACHE["names"] = {k: t.name for k, t in dram.items()}
    return nc, _CACHE["names"]


def _layernorm_np(x, w, b):
    mu = x.mean(-1, keepdims=True)
    var = ((x - mu) ** 2).mean(-1, keepdims=True)
    return (x - mu) / np.sqrt(var + EPS) * w + b


def _host_gate(q, k, ln_w, ln_b, w_in, wp_w1, wp_b1, wp_ln_w, wp_ln_b,
               wp_w2, wp_b2):
    """w[h, b] = sigmoid gate; needs only column-means of LN(q/k) @ w_in."""
    muq = _layernorm_np(q, ln_w, ln_b).mean(1) @ w_in   # [B, INNER]
    muk = _layernorm_np(k, ln_w, ln_b).mean(1) @ w_in
    fq = muq.reshape(B, H, DH).transpose(1, 0, 2)       # [H, B, DH]
    fk = muk.reshape(B, H, DH).transpose(1, 0, 2)
    feat = np.concatenate([fq, fk], axis=-1)            # [H, B, 2*DH]
    g = feat @ wp_w1 + wp_b1
    g = _layernorm_np(g, wp_ln_w, wp_ln_b)
    g = np.maximum(g, 0.0) @ wp_w2 + wp_b2              # [H, B, 1]
    return 1.0 / (1.0 + np.exp(-g[..., 0]))             # [H, B]


def kernel(q, k, v, ln_w, ln_b, w_in, wp_w1, wp_b1, wp_ln_w, wp_ln_b,
           wp_w2, wp_b2, w_out, b_out):
    q = np.asarray(q, np.float32)
    k = np.asarray(k, np.float32)
    v = np.asarray(v, np.float32)
    ln_w = np.asarray(ln_w, np.float32)
    ln_b = np.asarray(ln_b, np.float32)
    w_in = np.asarray(w_in, np.float32)
    w_out = np.asarray(w_out, np.float32)
    b_out = np.asarray(b_out, np.float32)
    assert not np.any(ln_b), "ln_b folding path assumes zero bias"

    w_gate = _host_gate(q, k, ln_w, ln_b, w_in,
                        np.asarray(wp_w1, np.float32), np.asarray(wp_b1, np.float32),
                        np.asarray(wp_ln_w, np.float32), np.asarray(wp_ln_b, np.float32),
                        np.asarray(wp_w2, np.float32), np.asarray(wp_b2, np.float32))

    W1 = (ln_w[:, None] * w_in).astype(ml_dtypes.bfloat16)     # [DIM, INNER]
    WO = w_out.astype(ml_dtypes.bfloat16)                       # [INNER, DIM]

    nc, names = _build()
    in_maps = []
    for core in range(NCORES):
        b, g = core // 2, core % 2
        w1g = W1[:, g * IG:(g + 1) * IG].reshape(8, 128, IG).transpose(1, 0, 2)
        wog = WO[g * IG:(g + 1) * IG, :].reshape(4, 128, DIM).transpose(1, 0, 2)
        scal = np.empty((128, HG), np.float32)
        for hl in range(HG):
            wg = w_gate[g * HG + hl, b]
            scal[0:DH, hl] = 1.0 - wg
            scal[DH:128, hl] = wg / DH
        in_maps.append({
            names["q"]: np.ascontiguousarray(q[b]),
            names["k"]: np.ascontiguousarray(k[b]),
            names["v"]: np.ascontiguousarray(v[b]),
            names["w1"]: np.ascontiguousarray(w1g),
            names["wout"]: np.ascontiguousarray(wog),
            names["scal"]: scal,
        })

    res = run_bass_kernel_spmd(nc, in_maps, core_ids=list(range(NCORES)))
    _CACHE["last_res"] = res
    out = np.empty((B, N, DIM), np.float32)
    for b in range(B):
        out[b] = res.results[2 * b][names["out"]] + res.results[2 * b + 1][names["out"]]
    out += b_out
    return out



# revision 16
# speedup vs baseline: 2.9593x; 2.9593x over previous
"""Trainium2 Bass kernel for nn_Attention_11106785428044.

Math (reference, per head h and batch b):
  f_x = LN(x) @ w_in (x in {q,k,v}), heads of size DH=64
  dots = (1-w)*cos_sim(f_q,f_k) + w*cov(f_q,f_k);  out = dots @ f_v; proj w_out
The attention matrix is used LINEARLY (no softmax), so with
  qhat = f_q/|f_q|, qc = f_q - mean_d(f_q), khat = f_k/|f_k|, kc = f_k - mean_d(f_k)
  dots @ f_v = [qhat, qc] @ G,  G = [(1-w)*khat, (w/DH)*kc]^T @ f_v   (G is 128x64)
which removes both NxN matrices. The gate w (a scalar per (h,b)) only needs
column-means of LN(q),LN(k) projected through w_in — computed on host.

LayerNorm is folded past the projection:  f = r*(x@W1) - (r*mu)*colsum(W1),
so the matmul runs on RAW bf16 x, which the HOST passes both token-major and
pre-transposed (feature-major) — plain wide DMAs, no PE transposes, no XBAR
for the big loads (XBAR descriptors are ~300B and saturate the DMA engines).
Row stats (mu, r) come from the token-major copy. Q2 -> QT (2MB total) does
use the XBAR SBUF->SBUF.

Sharding: 8 cores = 4 batches x 2 head-groups (8 heads / 512 inner dims each).
Each core computes a [1024,1024] partial of out = Y @ w_out; host sums the two
head-group partials per batch and adds b_out.
"""

import numpy as np
import ml_dtypes

import concourse.bass as bass
import concourse.mybir as mybir
import concourse.tile as tile
from concourse import bacc
from concourse.bass_utils import run_bass_kernel_spmd

F32 = mybir.dt.float32
BF16 = mybir.dt.bfloat16
AF = mybir.ActivationFunctionType
ALU = mybir.AluOpType

EPS = 1e-5
B, N, DIM = 4, 1024, 1024
H, DH = 16, 64
NT = 8          # token tiles of 128
HG = 8          # heads per core
IG = HG * DH    # inner dims per core = 512
NCORES = 8


def _emit(tc: tile.TileContext, dram):
    nc = tc.nc
    import contextlib
    ctx = contextlib.ExitStack()
    with ctx:
        consts = ctx.enter_context(tc.tile_pool(name="consts", bufs=1))
        xpool = ctx.enter_context(tc.tile_pool(name="x", bufs=3))
        xtpool = ctx.enter_context(tc.tile_pool(name="xt", bufs=2))
        fpool = ctx.enter_context(tc.tile_pool(name="f", bufs=1))
        ftmp = ctx.enter_context(tc.tile_pool(name="ftmp", bufs=2))
        kqpool = ctx.enter_context(tc.tile_pool(name="kq", bufs=1))
        q2pool = ctx.enter_context(tc.tile_pool(name="q2", bufs=2))
        stat = ctx.enter_context(tc.tile_pool(name="stat", bufs=6))
        gpool = ctx.enter_context(tc.tile_pool(name="g", bufs=1))
        opool = ctx.enter_context(tc.tile_pool(name="o", bufs=2))
        psF = ctx.enter_context(tc.tile_pool(name="psF", bufs=2, space="PSUM"))
        psG = ctx.enter_context(tc.tile_pool(name="psG", bufs=1, space="PSUM"))
        psY = ctx.enter_context(tc.tile_pool(name="psY", bufs=1, space="PSUM"))
        psO = ctx.enter_context(tc.tile_pool(name="psO", bufs=2, space="PSUM"))

        eps_sb = consts.tile([128, 1], F32)
        nc.vector.memset(eps_sb, EPS)

        # Per-queue DMA moves only ~0.2MB/us and transfer order = issue
        # order. Critical path for the first matmul: kT (sync queue, first),
        # w1 (parallel SWDGE queue), xb_k + csum (scalar queue).
        w1_sb = consts.tile([128, 8, IG], BF16)
        nc.gpsimd.dma_start(out=w1_sb, in_=dram["w1"])
        xts, xbs = {}, {}
        for tag in ("k", "v", "q"):
            xt = xtpool.tile([128, 8, N], BF16, tag="xt_" + tag)
            srcT = dram[tag + "T"].rearrange("(c p) n -> p c n", p=128)
            nc.sync.dma_start(out=xt[:, :, :], in_=srcT)
            xb = xpool.tile([128, NT, DIM], BF16, tag="xb_" + tag)
            src = dram[tag].rearrange("(t p) d -> p t d", p=128)
            nc.scalar.dma_start(out=xb[:, 0:4, :], in_=src[:, 0:4, :])
            nc.scalar.dma_start(out=xb[:, 4:8, :], in_=src[:, 4:8, :])
            xts[tag], xbs[tag] = xt, xb
            if tag == "k":
                csum_sb = consts.tile([128, IG], F32)
                nc.scalar.dma_start(out=csum_sb, in_=dram["csum"])
        wout_sb = consts.tile([128, 4, DIM], BF16)
        nc.sync.dma_start(out=wout_sb, in_=dram["wout"])
        scal_sb = consts.tile([128, HG], F32)
        nc.sync.dma_start(out=scal_sb, in_=dram["scal"])

        def ln_stats(xb, t):
            """row stats of raw x: rstd r and -r*mu (both [128,1] f32)."""
            st = stat.tile([128, 2, 6], F32, tag="st")
            nc.vector.bn_stats(st[:, 0, :], xb[:, t, 0:512])
            nc.vector.bn_stats(st[:, 1, :], xb[:, t, 512:1024])
            mv = stat.tile([128, 2], F32, tag="mv")
            nc.vector.bn_aggr(mv, st)
            rstd = stat.tile([128, 1], F32, tag="rstd")
            nc.scalar.activation(rstd, mv[:, 1:2], AF.Sqrt, bias=eps_sb)
            nc.vector.reciprocal(rstd, rstd)
            nrmu = stat.tile([128, 1], F32, tag="nrmu")
            nc.vector.tensor_scalar(
                nrmu, mv[:, 0:1], scalar1=rstd, scalar2=-1.0,
                op0=ALU.mult, op1=ALU.mult,
            )
            return rstd, nrmu

        def project_tile(xt, t):
            pf = psF.tile([128, IG], F32, tag="pf")
            for c in range(8):
                nc.tensor.matmul(
                    pf, lhsT=xt[:, c, t * 128:(t + 1) * 128], rhs=w1_sb[:, c, :],
                    start=(c == 0), stop=(c == 7),
                )
            return pf

        def fixup(f_dst, pf, rstd, nrmu):
            """f = r*(x@W1) + (-r*mu)*colsum(W1); scale on Act, rank-1 on
            DVE."""
            nc.scalar.mul(f_dst, pf, rstd)
            nc.vector.scalar_tensor_tensor(
                f_dst, csum_sb, nrmu, f_dst, op0=ALU.mult, op1=ALU.add
            )

        def head_stats(fv3):
            """per-token, per-head  1/||f|| and mean_d f  from bf16 f view."""
            sq = stat.tile([128, HG, DH], F32, tag="sq")
            nc.scalar.square(sq, fv3)
            ssq = stat.tile([128, HG], F32, tag="ssq")
            nc.vector.reduce_sum(ssq, sq, axis=mybir.AxisListType.X)
            s = stat.tile([128, HG], F32, tag="s")
            nc.vector.reduce_sum(s, fv3, axis=mybir.AxisListType.X)
            rn = stat.tile([128, HG], F32, tag="rn")
            nc.scalar.activation(rn, ssq, AF.Sqrt)
            nc.vector.reciprocal(rn, rn)
            m = stat.tile([128, HG], F32, tag="m")
            nc.scalar.mul(m, s, 1.0 / DH)
            return rn, m

        def form_pairs(hat_dst, c_dst, fv3, rn, m):
            """hat_dst = f*rn[h], c_dst = f - m[h]; head scalars broadcast
            across DH via stride-0 innermost dim. Both on Pool."""
            rn_b = rn[:, :, None].to_broadcast((128, HG, DH))
            m_b = m[:, :, None].to_broadcast((128, HG, DH))
            nc.gpsimd.tensor_tensor(hat_dst, fv3, rn_b, op=ALU.mult)
            nc.gpsimd.tensor_tensor(c_dst, fv3, m_b, op=ALU.subtract)

        # ---- K phase ----
        K2 = kqpool.tile([128, NT, HG, 2 * DH], BF16, tag="K2")
        for t in range(NT):
            rstd, nrmu = ln_stats(xbs["k"], t)
            pf = project_tile(xts["k"], t)
            f_k = ftmp.tile([128, IG], BF16, tag="fk")
            fixup(f_k, pf, rstd, nrmu)
            fv3 = f_k.rearrange("p (h d) -> p h d", h=HG)
            rn, m = head_stats(fv3)
            form_pairs(K2[:, t, :, 0:DH], K2[:, t, :, DH:2 * DH], fv3, rn, m)

        # ---- V phase ----
        f_v = fpool.tile([128, NT, IG], BF16, tag="fv")
        for t in range(NT):
            rstd, nrmu = ln_stats(xbs["v"], t)
            pf = project_tile(xts["v"], t)
            fixup(f_v[:, t, :], pf, rstd, nrmu)

        # ---- Q phase ----
        QT = kqpool.tile([128, HG, N], BF16, tag="QT")
        for t in range(NT):
            rstd, nrmu = ln_stats(xbs["q"], t)
            pf = project_tile(xts["q"], t)
            f_q = ftmp.tile([128, IG], BF16, tag="fq")
            fixup(f_q, pf, rstd, nrmu)
            fv3 = f_q.rearrange("p (h d) -> p h d", h=HG)
            rn, m = head_stats(fv3)
            q2 = q2pool.tile([128, HG, 2 * DH], BF16, tag="q2t")
            form_pairs(q2[:, :, 0:DH], q2[:, :, DH:2 * DH], fv3, rn, m)
            # feature-major QT via XBAR (SBUF->SBUF)
            nc.sync.dma_start_transpose(
                out=QT[:, :, t * 128:(t + 1) * 128],
                in_=q2.rearrange("p h d -> p (h d)"),
            )

        # ---- stage B: G_h = K2_h^T @ f_v_h (contract tokens); groups must be
        # consecutive per PSUM region (interleaved groups in one bank corrupt).
        # Emitted after Q so the in-order DVE/Act queues aren't blocked on it.
        pg = psG.tile([128, IG], F32, tag="pg")
        for h in range(HG):
            for t in range(NT):
                nc.tensor.matmul(
                    pg[:, h * DH:(h + 1) * DH],
                    lhsT=K2[:, t, h, :], rhs=f_v[:, t, h * DH:(h + 1) * DH],
                    start=(t == 0), stop=(t == NT - 1),
                )

        # scale G rows: [0:64]*(1-w), [64:128]*(w/DH) per head, one batched op
        G_sb = gpool.tile([128, HG, DH], BF16, tag="G")
        scal_b = scal_sb[:, :, None].to_broadcast((128, HG, DH))
        nc.vector.tensor_tensor(
            G_sb, pg.rearrange("p (h d) -> p h d", h=HG), scal_b, op=ALU.mult
        )

        # ---- stage C: Y^T_h = G_h^T @ Q''^T_h  -> feature-major Y^T ----
        # pack head pairs (2h, 2h+1) into partition halves of one PSUM tile
        YT = gpool.tile([128, 4, N], BF16, tag="YT")
        for hp in range(4):
            for blk in range(2):
                py = psY.tile([128, 512], F32, tag="py")
                for sub in range(2):
                    h = 2 * hp + sub
                    nc.tensor.matmul(
                        py[sub * DH:(sub + 1) * DH, :],
                        lhsT=G_sb[:, h, :],
                        rhs=QT[:, h, blk * 512:(blk + 1) * 512],
                        start=True, stop=True,
                    )
                nc.scalar.copy(YT[:, hp, blk * 512:(blk + 1) * 512], py)

        # ---- stage D: out = Y @ w_out  (contract inner), token-major out ----
        for t in range(NT):
            out_sb = opool.tile([128, DIM], F32, tag="osb")
            for cb in range(2):
                po = psO.tile([128, 512], F32, tag="po")
                for j in range(4):
                    nc.tensor.matmul(
                        po, lhsT=YT[:, j, t * 128:(t + 1) * 128],
                        rhs=wout_sb[:, j, cb * 512:(cb + 1) * 512],
                        start=(j == 0), stop=(j == 3),
                    )
                nc.scalar.copy(out_sb[:, cb * 512:(cb + 1) * 512], po)
            nc.scalar.dma_start(
                out=dram["out"][t * 128:(t + 1) * 128, :], in_=out_sb
            )


_CACHE = {}


def _build():
    if "nc" in _CACHE:
        return _CACHE["nc"], _CACHE["names"]
    nc = bacc.Bacc("TRN2", target_bir_lowering=False, debug=False)
    dram = {
        "q": nc.dram_tensor("q", [N, DIM], BF16, kind="ExternalInput"),
        "k": nc.dram_tensor("k", [N, DIM], BF16, kind="ExternalInput"),
        "v": nc.dram_tensor("v", [N, DIM], BF16, kind="ExternalInput"),
        "qT": nc.dram_tensor("qT", [DIM, N], BF16, kind="ExternalInput"),
        "kT": nc.dram_tensor("kT", [DIM, N], BF16, kind="ExternalInput"),
        "vT": nc.dram_tensor("vT", [DIM, N], BF16, kind="ExternalInput"),
        "w1": nc.dram_tensor("w1", [128, 8, IG], BF16, kind="ExternalInput"),
        "wout": nc.dram_tensor("wout", [128, 4, DIM], BF16, kind="ExternalInput"),
        "scal": nc.dram_tensor("scal", [128, HG], F32, kind="ExternalInput"),
        "csum": nc.dram_tensor("csum", [128, IG], F32, kind="ExternalInput"),
        "out": nc.dram_tensor("out", [N, DIM], F32, kind="ExternalOutput"),
    }
    with tile.TileContext(nc) as tc:
        _emit(tc, {k: v[:] for k, v in dram.items()})
    nc.compile()
    _CACHE["nc"] = nc
    _CACHE["names"] = {k: t.name for k, t in dram.items()}
    return nc, _CACHE["names"]


def _layernorm_np(x, w, b):
    mu = x.mean(-1, keepdims=True)
    var = ((x - mu) ** 2).mean(-1, keepdims=True)
    return (x - mu) / np.sqrt(var + EPS) * w + b


def _host_gate(q, k, ln_w, ln_b, w_in, wp_w1, wp_b1, wp_ln_w, wp_ln_b,
               wp_w2, wp_b2):
    """w[h, b] = sigmoid gate; needs only column-means of LN(q/k) @ w_in."""
    muq = _layernorm_np(q, ln_w, ln_b).mean(1) @ w_in   # [B, INNER]
    muk = _layernorm_np(k, ln_w, ln_b).mean(1) @ w_in
    fq = muq.reshape(B, H, DH).transpose(1, 0, 2)       # [H, B, DH]
    fk = muk.reshape(B, H, DH).transpose(1, 0, 2)
    feat = np.concatenate([fq, fk], axis=-1)            # [H, B, 2*DH]
    g = feat @ wp_w1 + wp_b1
    g = _layernorm_np(g, wp_ln_w, wp_ln_b)
    g = np.maximum(g, 0.0) @ wp_w2 + wp_b2              # [H, B, 1]
    return 1.0 / (1.0 + np.exp(-g[..., 0]))             # [H, B]


def kernel(q, k, v, ln_w, ln_b, w_in, wp_w1, wp_b1, wp_ln_w, wp_ln_b,
           wp_w2, wp_b2, w_out, b_out):
    q = np.asarray(q, np.float32)
    k = np.asarray(k, np.float32)
    v = np.asarray(v, np.float32)
    ln_w = np.asarray(ln_w, np.float32)
    ln_b = np.asarray(ln_b, np.float32)
    w_in = np.asarray(w_in, np.float32)
    w_out = np.asarray(w_out, np.float32)
    b_out = np.asarray(b_out, np.float32)
    assert not np.any(ln_b), "ln_b folding path assumes zero bias"

    w_gate = _host_gate(q, k, ln_w, ln_b, w_in,
                        np.asarray(wp_w1, np.float32), np.asarray(wp_b1, np.float32),
                        np.asarray(wp_ln_w, np.float32), np.asarray(wp_ln_b, np.float32),
                        np.asarray(wp_w2, np.float32), np.asarray(wp_b2, np.float32))

    W1 = (ln_w[:, None] * w_in).astype(ml_dtypes.bfloat16)     # [DIM, INNER]
    WO = w_out.astype(ml_dtypes.bfloat16)                       # [INNER, DIM]

    nc, names = _build()
    in_maps = []
    qb = q.astype(ml_dtypes.bfloat16)
    kb = k.astype(ml_dtypes.bfloat16)
    vb = v.astype(ml_dtypes.bfloat16)
    for core in range(NCORES):
        b, g = core // 2, core % 2
        w1g = W1[:, g * IG:(g + 1) * IG]
        w1c = w1g.reshape(8, 128, IG).transpose(1, 0, 2)
        wog = WO[g * IG:(g + 1) * IG, :].reshape(4, 128, DIM).transpose(1, 0, 2)
        csum = np.ascontiguousarray(np.broadcast_to(
            w1g.astype(np.float32).sum(0)[None, :], (128, IG)))
        scal = np.empty((128, HG), np.float32)
        for hl in range(HG):
            wg = w_gate[g * HG + hl, b]
            scal[0:DH, hl] = 1.0 - wg
            scal[DH:128, hl] = wg / DH
        in_maps.append({
            names["q"]: np.ascontiguousarray(qb[b]),
            names["k"]: np.ascontiguousarray(kb[b]),
            names["v"]: np.ascontiguousarray(vb[b]),
            names["qT"]: np.ascontiguousarray(qb[b].T),
            names["kT"]: np.ascontiguousarray(kb[b].T),
            names["vT"]: np.ascontiguousarray(vb[b].T),
            names["w1"]: np.ascontiguousarray(w1c),
            names["wout"]: np.ascontiguousarray(wog),
            names["scal"]: scal,
            names["csum"]: np.ascontiguousarray(csum),
        })

    res = run_bass_kernel_spmd(nc, in_maps, core_ids=list(range(NCORES)))
    _CACHE["last_res"] = res
    out = np.empty((B, N, DIM), np.float32)
    for b in range(B):
        out[b] = res.results[2 * b][names["out"]] + res.results[2 * b + 1][names["out"]]
    out += b_out
    return out
